# revision 5
# baseline (speedup 1.0000x reference)
"""Trainium2 Bass kernel for decayed event scatter-add (ExtractExclusivePatches).

Computes, for E events with sorted segment ids:
    out[n, k, c] = sum_{e: seg_e = n, kid_e = k} f_e[c] * exp(-(t_out[n] - dt_e) * rate_c)
with rate = softplus(decay_rate), out shape [N_OUT, K, C].

Strategy (8 NeuronCores, SPMD, no collectives):
  - Each core owns a contiguous range of output segments (N_OUT/8), i.e. a
    contiguous range of "flat slots" (flat = seg*K + kid, 225000 slots/core).
  - Host bins events by flat slot into per-core 128-slot windows and pads each
    window's event list to a fixed Kpad (uniform program across cores).
  - Device, per window: one DVE tensor_scalar builds a scaled one-hot matrix
    [Kpad events, 128 slots] = (iota == off) * g  where g = exp(-rate*elapsed)
    is the per-event decay (ACT engine); one matmul scatters the raw feature
    rows into a PSUM tile [128 slots, 64 ch]; ACT copies PSUM->SBUF staging;
    one DMA writes 8 windows (1024 slots) of contiguous output rows.
  - If rate is not channel-constant (decay_rate not constant), a general path
    computes per-event-per-channel decay on ACT and multiplies features on DVE.
"""

import math
import os

import numpy as np

# ---- problem constants (hardcoded per contract) ----
E_IN = 1_000_000
N_OUT = 200_000
C = 64
K = 9
NCORES = 8

SEGS_PER_CORE = N_OUT // NCORES          # 25000
SLOTS_PER_CORE = SEGS_PER_CORE * K       # 225000
W = 128                                   # slots per window (matmul M)
WPG = 8                                   # windows per group (psum banks / staging)
GROUPS = math.ceil(SLOTS_PER_CORE / (W * WPG))   # 220
WINDOWS = GROUPS * WPG                    # 1760
SGR = 8                                   # groups per scal DMA (64 windows)
SGROUPS = math.ceil(GROUPS / SGR)         # 28

_LN2 = float(np.log(2.0))


def _softplus(x):
    return np.logaddexp(0.0, x)


# ---------------------------------------------------------------- host side


def _preprocess(features, dt, times_out, successor_kernel_ids, segment_ids_out,
                decay_rate):
    """Bin events into per-core per-window padded streams."""
    seg = np.asarray(segment_ids_out, dtype=np.int64)
    kid = np.asarray(successor_kernel_ids, dtype=np.int64)
    flat = seg * K + kid                                    # [E] in [0, N_OUT*K)
    elapsed = (np.asarray(times_out, dtype=np.float32)[seg]
               - np.asarray(dt, dtype=np.float32))          # [E]

    core = flat // SLOTS_PER_CORE                           # [E] in [0,8)
    local = flat - core * SLOTS_PER_CORE
    w_local = local // W                                    # [0, 1758)
    off = (local - w_local * W).astype(np.float32)          # [0, 128)

    gw = core * WINDOWS + w_local                           # global window id
    order = np.argsort(gw, kind="stable")
    gw_s = gw[order]
    counts = np.bincount(gw_s, minlength=NCORES * WINDOWS)
    starts = np.concatenate([[0], np.cumsum(counts)[:-1]])
    rank = np.arange(E_IN, dtype=np.int64) - starts[gw_s]

    kpad = int(counts.max())
    assert kpad <= 128, f"window overflow: {kpad} events in one 128-slot window"
    # round up a little for DMA friendliness
    kpad = min(128, ((kpad + 3) // 4) * 4)

    # reorder per-event streams into sorted (core, window) order
    core_s = core[order]
    w_local_s = w_local[order]
    off_s = off[order]
    elapsed_s = elapsed[order]
    grp_s = w_local_s // WPG
    sub_s = w_local_s - grp_s * WPG

    # padded per-(core,window) feature stream: [NC, GROUPS, kpad, WPG, C]
    featw = np.zeros((NCORES * GROUPS * kpad * WPG, C), dtype=np.float32)
    dest = ((core_s * GROUPS + grp_s) * kpad + rank) * WPG + sub_s
    featw[dest] = np.asarray(features, dtype=np.float32)[order]
    featw = featw.reshape(NCORES, GROUPS, kpad, WPG * C)

    # scal stream: [NC, SGROUPS, kpad, SGR, 2, WPG]; j=0 -> elapsed, j=1 -> off
    scal = np.zeros((NCORES, SGROUPS, kpad, SGR, 2, WPG), dtype=np.float32)
    scal[:, :, :, :, 1, :] = -1.0                          # off=-1 -> no match
    sgrp_s = grp_s // SGR
    gg_s = grp_s - sgrp_s * SGR
    sdest = (((core_s * SGROUPS + sgrp_s) * kpad + rank) * SGR + gg_s) * 2 * WPG
    scal_flat = scal.reshape(-1)
    scal_flat[sdest + sub_s] = elapsed_s
    scal_flat[sdest + WPG + sub_s] = off_s
    scal = scal_flat.reshape(NCORES, SGROUPS, kpad, SGR * 2 * WPG)

    iota = np.tile(np.arange(W, dtype=np.float32), (128, 1))
    return featw, scal, iota, kpad


def _build_program(kpad, rate, groups=GROUPS, sgroups=SGROUPS, slots=None):
    """Build the Bass/Tile program (uniform across cores)."""
    import concourse.bacc as bacc
    import concourse.mybir as mybir
    import concourse.tile as tile

    rate = np.asarray(rate, dtype=np.float32)
    const_rate = bool(np.ptp(rate) <= 1e-12 * max(1.0, abs(float(rate[0]))))
    if slots is None:
        slots = groups * W * WPG

    nc = bacc.Bacc("TRN2", target_bir_lowering=False, debug=False,
                   enable_asserts=False)

    featw_d = nc.dram_tensor("featw", [groups, kpad, WPG * C], mybir.dt.float32,
                             kind="ExternalInput")
    scal_d = nc.dram_tensor("scal", [sgroups, kpad, SGR * 2 * WPG],
                            mybir.dt.float32, kind="ExternalInput")
    iota_d = nc.dram_tensor("iota", [128, W], mybir.dt.float32,
                            kind="ExternalInput")
    ratebc_d = None
    if not const_rate:
        ratebc_d = nc.dram_tensor("ratebc", [128, C], mybir.dt.float32,
                                  kind="ExternalInput")
    out_d = nc.dram_tensor("out", [slots, C], mybir.dt.float32,
                           kind="ExternalOutput")

    with tile.TileContext(nc) as tc:
        with (
            tc.tile_pool(name="const", bufs=1) as constp,
            tc.tile_pool(name="feats", bufs=3) as featp,
            tc.tile_pool(name="scal", bufs=2) as scalp,
            tc.tile_pool(name="work", bufs=4) as workp,
            tc.tile_pool(name="stage", bufs=3) as stagep,
            tc.tile_pool(name="psum", bufs=8, space="PSUM") as psump,
        ):
            iota_t = constp.tile([128, W], mybir.dt.float32)
            nc.sync.dma_start(out=iota_t[:], in_=iota_d.ap())
            ratebc_t = None
            if not const_rate:
                ratebc_t = constp.tile([128, C], mybir.dt.float32)
                nc.sync.dma_start(out=ratebc_t[:], in_=ratebc_d.ap())

            for sg in range(sgroups):
                scal_t = scalp.tile([kpad, SGR * 2 * WPG], mybir.dt.float32)
                nc.sync.dma_start(out=scal_t[:], in_=scal_d.ap()[sg])
                scal_v = scal_t[:].rearrange("p (g j w) -> p g j w", g=SGR, j=2)

                g_t = None
                if const_rate:
                    # g[e] = exp(-rate0 * elapsed[e]) for all 64 windows at once
                    g_t = workp.tile([kpad, SGR * WPG], mybir.dt.float32,
                                     tag="gdecay")
                    nc.scalar.activation(
                        out=g_t[:].rearrange("p (g w) -> p g w", g=SGR),
                        in_=scal_v[:, :, 0, :],
                        func=mybir.ActivationFunctionType.Exp,
                        scale=-float(rate[0]),
                    )

                for gg in range(min(SGR, groups - sg * SGR)):
                    grp = sg * SGR + gg
                    feat_t = featp.tile([kpad, WPG * C], mybir.dt.float32)
                    nc.sync.dma_start(out=feat_t[:], in_=featw_d.ap()[grp])
                    stage_t = stagep.tile([128, WPG * C], mybir.dt.float32)

                    for w in range(WPG):
                        off_col = scal_v[:, gg, 1, w:w + 1]
                        onehot_t = workp.tile([kpad, W], mybir.dt.float32,
                                              tag="onehot")
                        if const_rate:
                            nc.vector.tensor_scalar(
                                out=onehot_t[:],
                                in0=iota_t[:kpad, :],
                                scalar1=off_col,
                                scalar2=g_t[:, gg * WPG + w:gg * WPG + w + 1],
                                op0=mybir.AluOpType.is_equal,
                                op1=mybir.AluOpType.mult,
                            )
                            rhs = feat_t[:].rearrange(
                                "p (w c) -> p w c", w=WPG)[:, w, :]
                        else:
                            nc.vector.tensor_scalar(
                                out=onehot_t[:],
                                in0=iota_t[:kpad, :],
                                scalar1=off_col,
                                scalar2=None,
                                op0=mybir.AluOpType.is_equal,
                            )
                            decay_t = workp.tile([kpad, C], mybir.dt.float32,
                                                 tag="decay")
                            nc.scalar.activation(
                                out=decay_t[:],
                                in_=ratebc_t[:kpad, :],
                                func=mybir.ActivationFunctionType.Exp,
                                scale=scal_v[:, gg, 0, w:w + 1],
                            )
                            vals_t = workp.tile([kpad, C], mybir.dt.float32,
                                                tag="vals")
                            nc.vector.tensor_tensor(
                                out=vals_t[:],
                                in0=feat_t[:].rearrange(
                                    "p (w c) -> p w c", w=WPG)[:, w, :],
                                in1=decay_t[:],
                                op=mybir.AluOpType.mult,
                            )
                            rhs = vals_t[:]

                        psum_t = psump.tile([128, C], mybir.dt.float32,
                                            tag="acc")
                        nc.tensor.matmul(
                            out=psum_t[:],
                            lhsT=onehot_t[:],
                            rhs=rhs,
                            start=True,
                            stop=True,
                        )
                        nc.scalar.copy(
                            out=stage_t[:, w * C:(w + 1) * C], in_=psum_t[:])

                    nc.sync.dma_start(
                        out=out_d.ap()[grp * W * WPG:(grp + 1) * W * WPG]
                        .rearrange("(w p) c -> p w c", p=128),
                        in_=stage_t[:].rearrange("p (w c) -> p w c", w=WPG),
                    )
    nc.compile()
    return nc


def _run(nc, in_maps, **kwargs):
    from concourse import bass_utils
    return bass_utils.run_bass_kernel_spmd(
        nc, in_maps, core_ids=list(range(len(in_maps))), **kwargs)


def kernel(features, dt, times_out, successor_kernel_ids, segment_ids_out,
           decay_rate, _bench=None):
    features = np.asarray(features, dtype=np.float32)
    rate = _softplus(np.asarray(decay_rate, dtype=np.float32))

    featw, scal, iota, kpad = _preprocess(
        features, dt, times_out, successor_kernel_ids, segment_ids_out,
        decay_rate)

    nc = _build_program(kpad, rate)

    const_rate = bool(np.ptp(rate) <= 1e-12 * max(1.0, abs(float(rate[0]))))
    in_maps = []
    for c in range(NCORES):
        m = {"featw": featw[c], "scal": scal[c], "iota": iota}
        if not const_rate:
            m["ratebc"] = np.tile(-rate, (128, 1)).astype(np.float32)
        in_maps.append(m)

    if _bench is not None:
        res = _run(nc, in_maps, **_bench)
        outs = [r["out"] for r in res.results]
        full = np.concatenate([o[:SLOTS_PER_CORE] for o in outs], axis=0)
        return full.reshape(N_OUT, K, C), res

    res = _run(nc, in_maps)
    outs = [r["out"] for r in res.results]
    full = np.concatenate([o[:SLOTS_PER_CORE] for o in outs], axis=0)
    return full.reshape(N_OUT, K, C)
